# revision 15
# baseline (speedup 1.0000x reference)
"""Trainium2 Bass kernel for nn_DecoderLayer_50534585205086.

Sharding: 8 cores = 4 batches x 2 sequence halves. Each core processes
T=2048 tokens of one batch end-to-end (QKV proj + causal dwconv + avg-pool,
pooled causal attention, wup/wc projections, LN1+residual, squared-ReLU FFN,
LN2+residual). The only cross-core communication is a pairwise AllGather of
the pooled K/V (each seq half needs the other half's pooled keys/values for
causal attention; a 0/1 mask kills invalid key positions).

On-chip layout is feature-major ([channels, tokens]); the final output is
transposed back to token-major on the PE before writing out.

Matmuls run as float32r (full-rate fp32 PE mode; operands are produced by
rounding ops or casting DMAs as the BIR verifier requires); FFN2 runs in
bf16 to halve its weight/activation footprint.
"""

import numpy as np
from contextlib import ExitStack

import concourse.bass as bass
import concourse.tile as tile
from concourse import bacc, mybir
from concourse.bass import ts
from concourse.bass_utils import run_bass_kernel_spmd
from concourse.masks import make_identity

F32 = mybir.dt.float32
F32R = mybir.dt.float32r
BF16 = mybir.dt.bfloat16
AL = mybir.AluOpType
AF = mybir.ActivationFunctionType

N_CORES = 8
B, S_FULL, DM, H, DD, DF = 4, 4096, 1024, 16, 64, 4096
KER, KW = 4, 3
NORM = float(DD) ** -0.25
EPS = 1e-6
CT = DM // 128  # 8 channel tiles
FT = DF // 128  # 32 ffn tiles
HALO = 8


def _chunks(total, width):
    out = []
    c0 = 0
    while c0 < total:
        w = min(width, total - c0)
        out.append((c0, w))
        c0 += w
    return out


def build_program(S=S_FULL, mock_collective=False):
    T = S // 2           # tokens per core
    TH = T + HALO
    L = S // KER         # pooled length per batch
    LLOC = L // 2        # pooled positions owned per core
    MT = L // 128        # m tiles (keys)

    nc = bacc.Bacc("TRN2", target_bir_lowering=False, debug=False,
                   num_devices=N_CORES)

    def din(name, shape, dt=F32):
        return nc.dram_tensor(name, list(shape), dt, kind="ExternalInput").ap()

    xe_ap = din("xe", [DM, TH])
    xp_ap = din("xp", [DM, TH])
    wq_ap = din("wq", [CT, 128, DM], F32R)
    wk_ap = din("wk", [CT, 128, DM], F32R)
    wv_ap = din("wv", [CT, 128, DM], F32R)
    wc_ap = din("wc", [CT, 128, DM], F32R)
    w1_ap = din("w1", [FT, 128, DM], F32R)
    w2_ap = din("w2", [DF, DM])
    wup_ap = din("wup", [DD, DD], F32R)
    dwq_ap = din("dwq", [128, CT * KW])
    dwk_ap = din("dwk", [128, CT * KW])
    dwv_ap = din("dwv", [128, CT * KW])
    dbq_ap = din("dbq", [128, CT])
    dbk_ap = din("dbk", [128, CT])
    dbv_ap = din("dbv", [128, CT])
    bq_ap = din("bq", [128, CT])
    bk_ap = din("bk", [128, CT])
    bv_ap = din("bv", [128, CT])
    bc_ap = din("bc", [128, CT])
    b2_ap = din("b2", [128, CT])
    g1_ap = din("g1", [128, CT])
    be1_ap = din("be1", [128, CT])
    g2_ap = din("g2", [128, CT])
    be2_ap = din("be2", [128, CT])
    b1_ap = din("b1", [128, FT])
    bup_ap = din("bup", [128, 1])
    hm_ap = din("hmask", [128, HALO])
    mask_ap = din("mask", [L, LLOC])

    y_ap = nc.dram_tensor("y", [T, DM], F32, kind="ExternalOutput").ap()

    with tile.TileContext(nc) as tc, ExitStack() as ctx:
        const = ctx.enter_context(tc.tile_pool(name="const", bufs=1))
        dram = ctx.enter_context(tc.tile_pool(name="dram", bufs=1, space="DRAM"))

        kv_local = dram.tile([2, H, DD, LLOC], F32R, tag="kv_local")
        kv_all = dram.tile([2, 2, H, DD, LLOC], F32R, tag="kv_all")
        xemb_d = dram.tile([DM, T], F32R, tag="xemb_d")
        x1_d = dram.tile([DM, T], F32R, tag="x1_d")

        # ---- constants ----
        ident = const.tile([128, 128], F32, tag="ident")
        make_identity(nc, ident)
        ones_row_f = const.tile([1, 128], F32, tag="ones_row_f")
        nc.vector.memset(ones_row_f, 1.0)
        ones_row = const.tile([1, 128], F32R, tag="ones_row")
        nc.vector.tensor_copy(ones_row, ones_row_f)
        ones_col_f = const.tile([128, 1], F32, tag="ones_col_f")
        nc.vector.memset(ones_col_f, 1.0)
        ones_col = const.tile([128, 1], F32R, tag="ones_col")
        nc.vector.tensor_copy(ones_col, ones_col_f)
        eps_t = const.tile([1, 1], F32, tag="eps_t")
        nc.vector.memset(eps_t, EPS)
        wup_sb = const.tile([DD, DD], F32R, tag="wup_sb")
        nc.sync.dma_start(out=wup_sb, in_=wup_ap)
        ident_r = const.tile([128, 128], F32R, tag="ident_r")
        nc.vector.tensor_copy(ident_r, ident)
        bup_sb = const.tile([128, 1], F32, tag="bup_sb")
        nc.sync.dma_start(out=bup_sb, in_=bup_ap)
        hm_t = const.tile([128, HALO], F32, tag="hm_t")
        nc.sync.dma_start(out=hm_t, in_=hm_ap)

        def load_const(name, ap):
            t = const.tile(list(ap.shape), F32, tag=name, name=name)
            nc.sync.dma_start(out=t, in_=ap)
            return t

        bq_t = load_const("bq_t", bq_ap)
        bk_t = load_const("bk_t", bk_ap)
        bv_t = load_const("bv_t", bv_ap)
        bc_t = load_const("bc_t", bc_ap)
        b2_t = load_const("b2_t", b2_ap)
        g1_t = load_const("g1_t", g1_ap)
        be1_t = load_const("be1_t", be1_ap)
        g2_t = load_const("g2_t", g2_ap)
        be2_t = load_const("be2_t", be2_ap)
        b1_t = load_const("b1_t", b1_ap)

        bqn_t = const.tile([128, CT], F32, tag="bqn_t")
        nc.vector.tensor_scalar_mul(bqn_t, bq_t, NORM)
        bkn_t = const.tile([128, CT], F32, tag="bkn_t")
        nc.vector.tensor_scalar_mul(bkn_t, bk_t, NORM)

        # dwconv taps and biases pre-scaled by 1/KER (pooling mean folded in)
        taps = {}
        for nm, ap in (("q", dwq_ap), ("k", dwk_ap), ("v", dwv_ap)):
            t0 = load_const(f"dw{nm}_raw", ap)
            tsc = const.tile([128, CT * KW], F32, tag=f"dw{nm}_s")
            nc.vector.tensor_scalar_mul(tsc, t0, 1.0 / KER)
            taps[nm] = tsc
        dbs = {}
        for nm, ap in (("q", dbq_ap), ("k", dbk_ap), ("v", dbv_ap)):
            t0 = load_const(f"db{nm}_raw", ap)
            tsc = const.tile([128, CT], F32, tag=f"db{nm}_s")
            nc.vector.tensor_scalar_mul(tsc, t0, 1.0 / KER)
            dbs[nm] = tsc

        qp_ctx = ExitStack()
        qp_pool = qp_ctx.enter_context(tc.tile_pool(name="qp_pool", bufs=1, side="right"))
        qp_tiles = [qp_pool.tile([128, LLOC], F32R, tag=f"qp{i}", name=f"qp{i}")
                    for i in range(CT)]

        proj_chunks = _chunks(TH, 512)

        # ================= Stage A+B: embed, QKV proj, dwconv, pool ========
        with tc.tile_pool(name="sAB", bufs=1) as sab, \
             tc.tile_pool(name="psAB", bufs=1, space="PSUM") as psab:
            xemb_tiles = []
            for ci in range(CT):
                xs1 = sab.tile([128, TH], F32, tag="xs1", bufs=2, name=f"xs1_{ci}")
                nc.sync.dma_start(out=xs1, in_=xe_ap[ts(ci, 128), :])
                xs2 = sab.tile([128, TH], F32, tag="xs2", bufs=2, name=f"xs2_{ci}")
                nc.sync.dma_start(out=xs2, in_=xp_ap[ts(ci, 128), :])
                xm = sab.tile([128, TH], F32R, tag=f"xemb{ci}", name=f"xemb{ci}")
                nc.vector.tensor_add(xm, xs1, xs2)
                nc.sync.dma_start(out=xemb_d[ts(ci, 128), :], in_=xm[:, HALO:TH])
                xemb_tiles.append(xm)

            for kind, w_ap, bias_t, scale in (
                ("q", wq_ap, bqn_t, NORM),
                ("k", wk_ap, bkn_t, NORM),
                ("v", wv_ap, bv_t, 1.0),
            ):
                for co in range(CT):
                    wt = sab.tile([128, DM], F32R, tag="wblk", bufs=3,
                                  name=f"w{kind}{co}")
                    nc.sync.dma_start(out=wt, in_=w_ap[co])
                    wblks = [wt[:, ts(ci, 128)] for ci in range(CT)]
                    pre = sab.tile([128, TH], F32, tag="pre", bufs=2,
                                   name=f"pre{kind}{co}")
                    for (c0, cw) in proj_chunks:
                        ps = psab.tile([128, 512], F32, tag="qkv", bufs=3,
                                       name=f"ps{kind}{co}_{c0}")
                        for ci in range(CT):
                            nc.tensor.matmul(
                                ps[:, :cw], wblks[ci],
                                xemb_tiles[ci][:, c0:c0 + cw],
                                start=(ci == 0), stop=(ci == CT - 1))
                        nc.scalar.activation(pre[:, c0:c0 + cw], ps[:, :cw],
                                             AF.Identity,
                                             bias=bias_t[:, co:co + 1], scale=scale)
                    # reference zero-pads BEFORE dwconv/pooling: kill the
                    # bias-injected halo columns on first-half cores
                    nc.vector.tensor_mul(pre[:, 0:HALO], pre[:, 0:HALO], hm_t)
                    # causal depthwise conv (width 3), taps pre-scaled by 1/4
                    dw_t = taps[kind]
                    db_t = dbs[kind]
                    dw = sab.tile([128, TH - 2], F32, tag="dw", bufs=2,
                                  name=f"dw{kind}{co}")
                    nc.vector.tensor_scalar(dw, pre[:, 0:TH - 2],
                                            dw_t[:, co * KW:co * KW + 1],
                                            db_t[:, co:co + 1],
                                            op0=AL.mult, op1=AL.add)
                    nc.vector.scalar_tensor_tensor(
                        dw, pre[:, 1:TH - 1], dw_t[:, co * KW + 1:co * KW + 2], dw,
                        op0=AL.mult, op1=AL.add)
                    nc.vector.scalar_tensor_tensor(
                        dw, pre[:, 2:TH], dw_t[:, co * KW + 2:co * KW + 3], dw,
                        op0=AL.mult, op1=AL.add)
                    # pooling left-pad: windows must see zeros, not dwconv
                    # of bias terms, on first-half cores
                    nc.vector.tensor_mul(dw[:, 3:3 + KER - 1], dw[:, 3:3 + KER - 1],
                                         hm_t[:, 0:KER - 1])
                    # avg-pool win=stride=4 (mean folded into taps):
                    # out[l] = sum_j dw[4l+3+j], j=0..3   (dw index = token p-2)
                    dwv4 = dw[:, 3:3 + LLOC * KER].rearrange("p (l k) -> p k l",
                                                             k=KER)
                    sa = sab.tile([128, LLOC], F32, tag="poolsa", bufs=2,
                                  name=f"sa{kind}{co}")
                    nc.vector.tensor_add(sa, dwv4[:, 0, :], dwv4[:, 1, :])
                    sb_ = sab.tile([128, LLOC], F32, tag="poolsb", bufs=2,
                                   name=f"sb{kind}{co}")
                    nc.gpsimd.tensor_add(sb_, dwv4[:, 2, :], dwv4[:, 3, :])
                    if kind == "q":
                        nc.vector.tensor_add(qp_tiles[co], sa, sb_)
                    else:
                        kvp = sab.tile([128, LLOC], F32R, tag="kvp", bufs=3,
                                       name=f"kvp{kind}{co}")
                        nc.vector.tensor_add(kvp, sa, sb_)
                        kvi = 0 if kind == "k" else 1
                        nc.sync.dma_start(
                            out=kv_local[kvi, 2 * co:2 * co + 2].rearrange(
                                "h d m -> (h d) m"),
                            in_=kvp)

        # ================= Stage C: AllGather pooled K/V ====================
        if mock_collective:
            # timing-model variant: same traffic, no cross-core dependency
            nc.sync.dma_start(out=kv_all[0], in_=kv_local)
            nc.sync.dma_start(out=kv_all[1], in_=kv_local)
        else:
            nc.gpsimd.collective_compute(
                "AllGather", AL.bypass,
                replica_groups=[[0, 1], [2, 3], [4, 5], [6, 7]],
                ins=[kv_local.opt()], outs=[kv_all.opt()])

        # ============ Stage D: pooled causal attention + wup ================
        owup_ctx = ExitStack()
        with owup_ctx:
            owup_pool = owup_ctx.enter_context(
                tc.tile_pool(name="owup_pool", bufs=1))
            owup_tiles = [
                owup_pool.tile([128, T], F32R, tag=f"owup{i}", name=f"owup{i}")
                for i in range(CT)]

            with tc.tile_pool(name="sD", bufs=1) as sd, \
                 tc.tile_pool(name="psD", bufs=1, space="PSUM") as psd:
                mask_tiles = []
                for mt in range(MT):
                    m_t = sd.tile([128, LLOC], F32, tag=f"mask{mt}",
                                  name=f"mask{mt}")
                    nc.sync.dma_start(out=m_t, in_=mask_ap[ts(mt, 128), :])
                    mask_tiles.append(m_t)

                for hp in range(H // 2):
                    # kp for the head pair: rows 0-63 = head 2hp, 64-127 = 2hp+1
                    kp2 = sd.tile([128, L], F32R, tag="kp2", bufs=2,
                                  name=f"kp2_{hp}")
                    nc.sync.dma_start(
                        out=kp2,
                        in_=kv_all[:, 0, 2 * hp:2 * hp + 2].rearrange(
                            "g h d m -> (h d) g m"))

                    recs = []
                    osbs = []
                    for j in range(2):
                        h = 2 * hp + j
                        vp_h = sd.tile([DD, L], F32R, tag="vph", bufs=2,
                                       name=f"vp{h}")
                        nc.sync.dma_start(
                            out=vp_h,
                            in_=kv_all[:, 1, h].rearrange("g d m -> d g m"))

                        qp_h = qp_tiles[hp][j * DD:(j + 1) * DD, :]
                        kp_h = kp2[j * DD:(j + 1) * DD, :]

                        ps_av = psd.tile([DD, LLOC], F32, tag="av", bufs=2,
                                         name=f"av{h}")
                        ps_den = psd.tile([1, LLOC], F32, tag="den", bufs=1,
                                          name=f"den{h}")
                        for mt in range(MT):
                            ps_lg = psd.tile([128, LLOC], F32, tag="lg", bufs=2,
                                             name=f"lg{h}_{mt}")
                            nc.tensor.matmul(ps_lg, kp_h[:, ts(mt, 128)], qp_h,
                                             start=True, stop=True,
                                             tile_position=(j * DD, 0))
                            wexp = sd.tile([128, LLOC], F32, tag="wexp", bufs=3,
                                           name=f"wexp{h}_{mt}")
                            nc.scalar.activation(wexp, ps_lg, AF.Exp)
                            wexpm = sd.tile([128, LLOC], F32R, tag="wexpm",
                                            bufs=MT + 2, name=f"wexpm{h}_{mt}")
                            nc.vector.tensor_mul(wexpm, wexp, mask_tiles[mt])

                            ps_tr = psd.tile([128, DD], F32R, tag="tr", bufs=1,
                                             name=f"tr{h}_{mt}")
                            nc.tensor.transpose(ps_tr, vp_h[:, ts(mt, 128)],
                                                ident_r[0:DD, 0:DD])
                            vpt = sd.tile([128, DD], F32R, tag="vpt", bufs=3,
                                          name=f"vpt{h}_{mt}")
                            nc.scalar.copy(vpt, ps_tr.bitcast(F32))

                            nc.tensor.matmul(ps_av, vpt, wexpm,
                                             start=(mt == 0), stop=(mt == MT - 1))
                            nc.tensor.matmul(ps_den, ones_col, wexpm,
                                             start=(mt == 0), stop=(mt == MT - 1))

                        o_sb = sd.tile([DD, LLOC], F32R, tag="osb", bufs=2,
                                       name=f"osb{h}")
                        nc.scalar.copy(o_sb, ps_av)
                        rec = sd.tile([1, LLOC], F32R, tag="rec", bufs=2,
                                      name=f"rec{h}")
                        with nc.allow_low_precision(reason="f32r denom recip"):
                            nc.vector.reciprocal(rec, ps_den)
                        recs.append(rec)
                        osbs.append(o_sb)

                    # per-head post-processing (f32r matmuls must write
                    # psum at base partition 0; upsample copies shift rows)
                    up4 = owup_tiles[hp].rearrange("p (l k) -> p l k", k=KER)
                    for j in range(2):
                        ps_o2 = psd.tile([DD, LLOC], F32, tag="o2", bufs=1,
                                         name=f"o2_{hp}_{j}")
                        nc.tensor.matmul(ps_o2, wup_sb, osbs[j],
                                         start=True, stop=True)
                        ps_bc = psd.tile([DD, LLOC], F32, tag="bc", bufs=1,
                                         name=f"bc{hp}_{j}")
                        nc.tensor.matmul(ps_bc, ones_row[0:1, 0:DD], recs[j],
                                         start=True, stop=True)
                        bc_sb = sd.tile([DD, LLOC], F32, tag="bcs", bufs=2,
                                        name=f"bcs{hp}_{j}")
                        nc.scalar.copy(bc_sb, ps_bc)
                        own = sd.tile([DD, LLOC], F32, tag="own", bufs=2,
                                      name=f"own{hp}_{j}")
                        nc.vector.tensor_mul(own, ps_o2, bc_sb)
                        nc.scalar.activation(own, own, AF.Identity,
                                             bias=bup_sb[0:DD, 0:1], scale=1.0)
                        # upsample x4 into resident owup (feature-major)
                        for r in range(KER):
                            nc.scalar.copy(up4[j * DD:(j + 1) * DD, :, r], own)

            qp_ctx.close()

            # ============ Stage E: wc proj + LN1 + residual -> x1 ===========
            with tc.tile_pool(name="sE", bufs=1) as se, \
                 tc.tile_pool(name="psE", bufs=1, space="PSUM") as pse:
                for (c0, cw) in _chunks(T, 512):
                    xec = []
                    for ci in range(CT):
                        xt = se.tile([128, 512], F32R, tag="xec", bufs=CT + 1,
                                     name=f"xec{ci}_{c0}")
                        nc.sync.dma_start(out=xt,
                                          in_=xemb_d[ts(ci, 128), c0:c0 + cw])
                        xec.append(xt)

                    ps_s1 = pse.tile([1, 512], F32, tag="s1", bufs=1,
                                     name=f"s1_{c0}")
                    ps_s2 = pse.tile([1, 512], F32, tag="s2", bufs=1,
                                     name=f"s2_{c0}")
                    a_tiles = []
                    for co in range(CT):
                        wct = se.tile([128, DM], F32R, tag="wcb", bufs=3,
                                      name=f"wcb{co}_{c0}")
                        nc.sync.dma_start(out=wct, in_=wc_ap[co])
                        wcb = [wct[:, ts(ci, 128)] for ci in range(CT)]
                        ps_wc = pse.tile([128, 512], F32, tag="wc", bufs=2,
                                         name=f"pswc{co}_{c0}")
                        for ci in range(CT):
                            nc.tensor.matmul(ps_wc, wcb[ci],
                                             owup_tiles[ci][:, c0:c0 + cw],
                                             start=(ci == 0), stop=(ci == CT - 1))
                        a_sb = se.tile([128, 512], F32R, tag="asb", bufs=CT + 1,
                                       name=f"asb{co}_{c0}")
                        nc.scalar.activation(a_sb, ps_wc, AF.Identity,
                                             bias=bc_t[:, co:co + 1], scale=1.0)
                        a2 = se.tile([128, 512], F32R, tag="a2", bufs=2,
                                     name=f"a2_{co}_{c0}")
                        nc.vector.tensor_mul(a2, a_sb.bitcast(F32),
                                             a_sb.bitcast(F32))
                        nc.tensor.matmul(ps_s1, ones_col, a_sb,
                                         start=(co == 0), stop=(co == CT - 1))
                        nc.tensor.matmul(ps_s2, ones_col, a2,
                                         start=(co == 0), stop=(co == CT - 1))
                        a_tiles.append(a_sb)

                    mean = se.tile([1, 512], F32R, tag="mean", bufs=1,
                                   name=f"mean{c0}")
                    nc.vector.tensor_scalar_mul(mean, ps_s1, 1.0 / DM)
                    e2 = se.tile([1, 512], F32, tag="e2", bufs=1,
                                 name=f"e2_{c0}")
                    nc.vector.tensor_scalar_mul(e2, ps_s2, 1.0 / DM)
                    m2 = se.tile([1, 512], F32, tag="m2", bufs=1,
                                 name=f"m2_{c0}")
                    nc.vector.tensor_mul(m2, mean.bitcast(F32),
                                         mean.bitcast(F32))
                    var = se.tile([1, 512], F32, tag="var", bufs=1,
                                  name=f"var{c0}")
                    nc.vector.tensor_sub(var, e2, m2)
                    sd_t = se.tile([1, 512], F32, tag="sd", bufs=1,
                                   name=f"sd{c0}")
                    nc.scalar.activation(sd_t, var, AF.Sqrt,
                                         bias=eps_t[0:1, 0:1])
                    rstd = se.tile([1, 512], F32R, tag="rstd", bufs=1,
                                   name=f"rstd{c0}")
                    with nc.allow_low_precision(reason="f32r rstd"):
                        nc.vector.reciprocal(rstd, sd_t)

                    ps_mb = pse.tile([128, 512], F32, tag="ebc", bufs=2,
                                     name=f"mb{c0}")
                    nc.tensor.matmul(ps_mb, ones_row, mean,
                                     start=True, stop=True)
                    mbc = se.tile([128, 512], F32, tag="mbc", bufs=2,
                                  name=f"mbc{c0}")
                    nc.scalar.copy(mbc, ps_mb)
                    ps_rb = pse.tile([128, 512], F32, tag="ebc", bufs=2,
                                     name=f"rb{c0}")
                    nc.tensor.matmul(ps_rb, ones_row, rstd,
                                     start=True, stop=True)
                    rbc = se.tile([128, 512], F32, tag="rbc", bufs=2,
                                  name=f"rbc{c0}")
                    nc.scalar.copy(rbc, ps_rb)

                    for co in range(CT):
                        v1 = se.tile([128, 512], F32, tag="lnv", bufs=2,
                                     name=f"lnv{co}_{c0}")
                        nc.vector.tensor_sub(v1, a_tiles[co].bitcast(F32), mbc)
                        v2 = se.tile([128, 512], F32, tag="lnu", bufs=2,
                                     name=f"lnu{co}_{c0}")
                        nc.gpsimd.tensor_mul(v2, v1, rbc)
                        v3 = se.tile([128, 512], F32, tag="lnw", bufs=2,
                                     name=f"lnw{co}_{c0}")
                        nc.vector.tensor_scalar(v3, v2, g1_t[:, co:co + 1],
                                                be1_t[:, co:co + 1],
                                                op0=AL.mult, op1=AL.add)
                        x1c = se.tile([128, 512], F32R, tag="x1c", bufs=2,
                                      name=f"x1c{co}_{c0}")
                        nc.vector.tensor_add(x1c, v3, xec[co].bitcast(F32))
                        nc.sync.dma_start(out=x1_d[ts(co, 128), c0:c0 + cw],
                                          in_=x1c)

        # ======== Stage F: FFN + LN2 + residual + output transpose ==========
        with tc.tile_pool(name="sF", bufs=1) as sf, \
             tc.tile_pool(name="psF", bufs=1, space="PSUM") as psf:
            # cast w2 to bf16 once (resident)
            w2bf_tiles = []
            for f in range(FT):
                w2b = sf.tile([128, DM], F32, tag="w2b", bufs=2, name=f"w2b{f}")
                nc.sync.dma_start(out=w2b, in_=w2_ap[ts(f, 128), :])
                w2bf = sf.tile([128, DM], BF16, tag=f"w2bf{f}", name=f"w2bf{f}")
                nc.scalar.copy(w2bf, w2b)
                w2bf_tiles.append(w2bf)

            for (c0, cw) in _chunks(T, 512):
                x1f = []
                for ci in range(CT):
                    xt = sf.tile([128, 512], F32R, tag="x1f", bufs=CT + 1,
                                 name=f"x1f{ci}_{c0}")
                    nc.sync.dma_start(out=xt, in_=x1_d[ts(ci, 128), c0:c0 + cw])
                    x1f.append(xt)

                hb_tiles = []
                for f in range(FT):
                    w1t = sf.tile([128, DM], F32R, tag="w1b", bufs=3,
                                  name=f"w1t{f}_{c0}")
                    nc.sync.dma_start(out=w1t, in_=w1_ap[f])
                    w1blks = [w1t[:, ts(ci, 128)] for ci in range(CT)]
                    ps_h = psf.tile([128, 512], F32, tag="fps", bufs=2,
                                    name=f"psh{f}_{c0}")
                    for ci in range(CT):
                        nc.tensor.matmul(ps_h, w1blks[ci], x1f[ci],
                                         start=(ci == 0), stop=(ci == CT - 1))
                    hr = sf.tile([128, 512], F32, tag="hr", bufs=2,
                                 name=f"hr{f}_{c0}")
                    nc.scalar.activation(hr, ps_h, AF.Relu,
                                         bias=b1_t[:, f:f + 1], scale=1.0)
                    hb = sf.tile([128, 512], BF16, tag=f"hb{f}", name=f"hb{f}_{c0}")
                    nc.vector.tensor_mul(hb, hr, hr)
                    hb_tiles.append(hb)

                ps_s1 = psf.tile([1, 512], F32, tag="fs1", bufs=1,
                                 name=f"fs1_{c0}")
                ps_s2 = psf.tile([1, 512], F32, tag="fs2", bufs=1,
                                 name=f"fs2_{c0}")
                ffw_tiles = []
                for co in range(CT):
                    ps_y = psf.tile([128, 512], F32, tag="yps", bufs=2,
                                    name=f"psy{co}_{c0}")
                    for f in range(FT):
                        nc.tensor.matmul(ps_y, w2bf_tiles[f][:, ts(co, 128)],
                                         hb_tiles[f],
                                         start=(f == 0), stop=(f == FT - 1))
                    ffw = sf.tile([128, 512], F32R, tag="fsb", bufs=CT + 1,
                                  name=f"ffw{co}_{c0}")
                    nc.scalar.activation(ffw, ps_y, AF.Identity,
                                         bias=b2_t[:, co:co + 1], scale=1.0)
                    f2 = sf.tile([128, 512], F32R, tag="f2", bufs=2,
                                 name=f"f2_{co}_{c0}")
                    nc.vector.tensor_mul(f2, ffw.bitcast(F32), ffw.bitcast(F32))
                    nc.tensor.matmul(ps_s1, ones_col, ffw,
                                     start=(co == 0), stop=(co == CT - 1))
                    nc.tensor.matmul(ps_s2, ones_col, f2,
                                     start=(co == 0), stop=(co == CT - 1))
                    ffw_tiles.append(ffw)

                mean = sf.tile([1, 512], F32R, tag="fmean", bufs=1,
                               name=f"fmean{c0}")
                nc.vector.tensor_scalar_mul(mean, ps_s1, 1.0 / DM)
                e2 = sf.tile([1, 512], F32, tag="fe2", bufs=1, name=f"fe2_{c0}")
                nc.vector.tensor_scalar_mul(e2, ps_s2, 1.0 / DM)
                m2 = sf.tile([1, 512], F32, tag="fm2", bufs=1, name=f"fm2_{c0}")
                nc.vector.tensor_mul(m2, mean.bitcast(F32), mean.bitcast(F32))
                var = sf.tile([1, 512], F32, tag="fvar", bufs=1,
                              name=f"fvar{c0}")
                nc.vector.tensor_sub(var, e2, m2)
                sd_t = sf.tile([1, 512], F32, tag="fsd", bufs=1, name=f"fsd{c0}")
                nc.scalar.activation(sd_t, var, AF.Sqrt, bias=eps_t[0:1, 0:1])
                rstd = sf.tile([1, 512], F32R, tag="frstd", bufs=1,
                               name=f"frstd{c0}")
                with nc.allow_low_precision(reason="f32r rstd"):
                    nc.vector.reciprocal(rstd, sd_t)

                ps_mb = psf.tile([128, 512], F32, tag="fbc", bufs=1,
                                 name=f"fmb{c0}")
                nc.tensor.matmul(ps_mb, ones_row, mean, start=True, stop=True)
                mbc = sf.tile([128, 512], F32, tag="fmbc", bufs=2,
                              name=f"fmbc{c0}")
                nc.scalar.copy(mbc, ps_mb)
                ps_rb = psf.tile([128, 512], F32, tag="fbc", bufs=1,
                                 name=f"frb{c0}")
                nc.tensor.matmul(ps_rb, ones_row, rstd, start=True, stop=True)
                rbc = sf.tile([128, 512], F32, tag="frbc", bufs=2,
                              name=f"frbc{c0}")
                nc.scalar.copy(rbc, ps_rb)

                for co in range(CT):
                    v1 = sf.tile([128, 512], F32, tag="flnv", bufs=2,
                                 name=f"flnv{co}_{c0}")
                    nc.vector.tensor_sub(v1, ffw_tiles[co].bitcast(F32), mbc)
                    v2 = sf.tile([128, 512], F32, tag="flnu", bufs=2,
                                 name=f"flnu{co}_{c0}")
                    nc.gpsimd.tensor_mul(v2, v1, rbc)
                    v3 = sf.tile([128, 512], F32, tag="flnw", bufs=2,
                                 name=f"flnw{co}_{c0}")
                    nc.vector.tensor_scalar(v3, v2, g2_t[:, co:co + 1],
                                            be2_t[:, co:co + 1],
                                            op0=AL.mult, op1=AL.add)
                    yfm = sf.tile([128, 512], F32, tag="yfm", bufs=2,
                                  name=f"yfm{co}_{c0}")
                    nc.vector.tensor_add(yfm, v3, x1f[co].bitcast(F32))
                    # transpose to token-major and write out
                    for tb in range(4):
                        ps_t = psf.tile([128, 128], F32, tag="ytr", bufs=1,
                                        name=f"ytr{co}_{tb}_{c0}")
                        nc.tensor.transpose(ps_t, yfm[:, ts(tb, 128)], ident)
                        ytb = sf.tile([128, 128], F32, tag="ytb", bufs=4,
                                      name=f"ytb{co}_{tb}_{c0}")
                        nc.scalar.copy(ytb, ps_t)
                        nc.sync.dma_start(
                            out=y_ap[c0 + tb * 128:c0 + (tb + 1) * 128,
                                     ts(co, 128)],
                            in_=ytb)

    nc.compile()
    return nc


_PROGRAM_CACHE = {}


def _get_program(S=S_FULL):
    if S not in _PROGRAM_CACHE:
        _PROGRAM_CACHE[S] = build_program(S)
    return _PROGRAM_CACHE[S]


def _vec_fold(v, cols):
    """[N] -> [128, N//128] with column i = v[i*128:(i+1)*128]."""
    v = np.asarray(v, np.float32)
    return np.ascontiguousarray(v.reshape(cols, 128).T)


def prep_inputs(inputs, S=S_FULL):
    T = S // 2
    L = S // KER
    LLOC = L // 2

    g = {k: np.asarray(v, np.float32) for k, v in inputs.items()}

    def wtile(w, nt):
        ci = w.shape[0] // 128
        return np.ascontiguousarray(
            w.reshape(ci, 128, nt, 128).transpose(2, 1, 0, 3)
            .reshape(nt, 128, ci * 128))

    shared = {
        "wq": wtile(g["wq"], CT), "wk": wtile(g["wk"], CT),
        "wv": wtile(g["wv"], CT), "wc": wtile(g["wc"], CT),
        "w1": wtile(g["w1"], FT), "w2": g["w2"], "wup": g["wup"],
        "bq": _vec_fold(g["bq"], CT), "bk": _vec_fold(g["bk"], CT),
        "bv": _vec_fold(g["bv"], CT), "bc": _vec_fold(g["bc"], CT),
        "b2": _vec_fold(g["b2"], CT), "g1": _vec_fold(g["g1"], CT),
        "be1": _vec_fold(g["be1"], CT), "g2": _vec_fold(g["g2"], CT),
        "be2": _vec_fold(g["be2"], CT), "b1": _vec_fold(g["b1"], FT),
        "dbq": _vec_fold(g["dbq"], CT), "dbk": _vec_fold(g["dbk"], CT),
        "dbv": _vec_fold(g["dbv"], CT),
        "bup": np.ascontiguousarray(np.tile(g["bup"].reshape(DD), 2).reshape(128, 1)),
    }
    for nm in ("dwq", "dwk", "dwv"):
        w = g[nm]  # [3, DM]
        shared[nm] = np.ascontiguousarray(
            w.T.reshape(CT, 128, KW).transpose(1, 0, 2).reshape(128, CT * KW))

    in_maps = []
    for c in range(N_CORES):
        b, hf = c // 2, c % 2
        m = dict(shared)
        for nm, arr in (("xe", g["x_enc"]), ("xp", g["x_pos"])):
            fm = arr[b].T  # [DM, S]
            if hf == 0:
                sl = np.concatenate(
                    [np.zeros((DM, HALO), np.float32), fm[:, :T]], axis=1)
            else:
                sl = fm[:, T - HALO:2 * T]
            m[nm] = np.ascontiguousarray(sl)
        m["hmask"] = np.full((128, HALO), float(hf), np.float32)
        m["mask"] = np.ascontiguousarray(
            (np.arange(L)[:, None] <= (hf * LLOC + np.arange(LLOC))[None, :])
            .astype(np.float32))
        in_maps.append(m)
    return in_maps


def gather_output(results, S=S_FULL):
    T = S // 2
    y = np.empty((B, S, DM), np.float32)
    for c in range(N_CORES):
        b, hf = c // 2, c % 2
        y[b, hf * T:(hf + 1) * T, :] = results[c]["y"]
    return y


def kernel(**inputs):
    nc = _get_program(S_FULL)
    in_maps = prep_inputs(inputs, S_FULL)
    res = run_bass_kernel_spmd(nc, in_maps, list(range(N_CORES)))
    return gather_output(res.results, S_FULL)
